# revision 1
# baseline (speedup 1.0000x reference)
"""ActiveRotatingFilter gather kernel for 8 Trainium2 NeuronCores.

Semantics (matching the reference):
    idx = indices.reshape(72, 8) - 1
    inv = argsort(idx, axis=0)   (stable)
    out[o, r, i, e] = input[o, i, inv[e, r]]      out: [O*R, I*nOri, kH, kW]

Strategy: shard O=512 across 8 cores (64 planes each). Per core the input
shard (4.7 MB) is loaded once into SBUF as [128 partitions = (o, i_hi),
9216 = (i_lo, e)]. For each of the 8 rotations the 72-entry permutation is
applied on-chip by VectorE copies (the ARF permutation factors into a
cyclic layer shift + a 9-element kernel permutation, giving <=18 strided
block copies per rotation; identity rotations skip the copy entirely),
then the permuted tile is written out with a fully-contiguous 4.7 MB DMA
per rotation.

The 16 SDMA engines sustain ~27 GB/s each (~433 GB/s/core aggregate,
shared between reads and writes), so the roofline is total traffic:
(4.7 read + 37.75 write) MB ~= 98 us of DMA streaming plus ~13 us of
fixed NEFF preamble/tail. Reads go on the scalar-engine HWDGE ring in C
chunks along i_lo; all output writes go on the sync-engine ring.
Identity-rotation output chunks and the first copy-rotation's VectorE
copies are gated per input chunk so the write stream starts as early as
possible and never stalls; VectorE's ~75 us of permute copies hide
entirely under the DMA stream.
"""

import numpy as np
from contextlib import ExitStack

O, I, NORI, KH, KW = 512, 256, 8, 3, 3
R = 8
E = NORI * KH * KW          # 72
NCORES = 8
O_SH = O // NCORES          # 64 output planes per core
P = 128                     # SBUF partitions, p = o*2 + i_hi
IL = I // 2                 # 128 i_lo values per partition
FD = IL * E                 # 9216 f32 per partition
NB = 3                      # y-tile ring buffers
C = 2                       # input chunks (along i_lo)
ILC = IL // C               # 64 i_lo per chunk
FDC = ILC * E               # 4608 free elems per chunk

_cache = {}


def _plan_rotation(col):
    """Decompose one permutation column into block-copy ops.

    Returns a list of ops:
      ("lgroup", s, j, qj): for all l: dst (l, j) <- src ((l - s) % 8, qj)
      ("run", a, b, ln):    dst [a, a+ln) <- src [b, b+ln)
    """
    col = col.astype(int)
    layers = col.reshape(NORI, KH * KW) // (KH * KW)
    q = col.reshape(NORI, KH * KW) % (KH * KW)
    structured = all(np.all(layers[l] == layers[l][0]) for l in range(NORI))
    if structured:
        l0 = layers[:, 0]
        s = int((-l0[0]) % NORI)
        structured = np.array_equal(l0, (np.arange(NORI) - s) % NORI) and all(
            np.array_equal(q[l], q[0]) for l in range(NORI)
        )
    if structured:
        return [("lgroup", s, j, int(q[0][j])) for j in range(KH * KW)]
    ops = []
    e = 0
    while e < E:
        b = int(col[e])
        ln = 1
        while e + ln < E and col[e + ln] == b + ln:
            ln += 1
        ops.append(("run", e, b, ln))
        e += ln
    return ops


def _emit_rotation_copies(vector, rot_plan, x_t, yt, cp_sem, il_lo, il_hi, last):
    """Emit VectorE copies for one rotation, restricted to i_lo in [il_lo, il_hi)."""
    x4 = x_t[:].rearrange("p (il l j) -> p il l j", il=IL, l=NORI)
    y4 = yt[:].rearrange("p (il l j) -> p il l j", il=IL, l=NORI)
    x3 = x_t[:].rearrange("p (il e) -> p il e", il=IL)
    y3 = yt[:].rearrange("p (il e) -> p il e", il=IL)
    sl = slice(il_lo, il_hi)
    pairs = []
    for op in rot_plan:
        if op[0] == "lgroup":
            _, s, j, qj = op
            if s == 0:
                pairs.append((y4[:, sl, :, j], x4[:, sl, :, qj]))
            else:
                pairs.append((y4[:, sl, s:NORI, j], x4[:, sl, 0 : NORI - s, qj]))
                pairs.append((y4[:, sl, 0:s, j], x4[:, sl, NORI - s : NORI, qj]))
        else:
            _, a, b, ln = op
            pairs.append((y3[:, sl, a : a + ln], x3[:, sl, b : b + ln]))
    for i, (dst, src) in enumerate(pairs):
        instr = vector.tensor_copy(dst, src)
        if last and i == len(pairs) - 1:
            instr.then_inc(cp_sem, 1)


def _build(inv):
    import concourse.bass as bass
    import concourse.mybir as mybir

    f32 = mybir.dt.float32
    nc = bass.Bass("TRN2", target_bir_lowering=False, debug=False)
    x = nc.declare_dram_parameter("input", [P, FD], f32, isOutput=False)
    out = nc.declare_dram_parameter("out", [O_SH, R, 2, FD], f32, isOutput=True)

    ident = [r for r in range(R) if np.array_equal(inv[:, r], np.arange(E))]
    copies = [r for r in range(R) if r not in ident]
    n_id = len(ident)
    n_cp = len(copies)
    rot_plans = {r: _plan_rotation(inv[:, r]) for r in copies}
    n_wr = n_id * C + n_cp  # total output DMAs

    with ExitStack() as ctx:
        x_t = ctx.enter_context(nc.sbuf_tensor("x_t", [P, FD], f32))
        y_t = [
            ctx.enter_context(nc.sbuf_tensor(f"y_t{b}", [P, FD], f32))
            for b in range(NB)
        ]
        rd_sem = ctx.enter_context(nc.semaphore("rd_sem"))
        wr_sem = ctx.enter_context(nc.semaphore("wr_sem"))
        cp_sem = ctx.enter_context(nc.semaphore("cp_sem"))
        block = ctx.enter_context(nc.Block())

        @block.scalar
        def _(scalar):
            # input load, C chunks along the free (i_lo) dim — read stream
            for c in range(C):
                fsl = slice(c * FDC, (c + 1) * FDC)
                scalar.dma_start(x_t[:, fsl], x[:, fsl]).then_inc(rd_sem, 16)
            scalar.wait_ge(rd_sem, 16 * C)

        @block.sync
        def _(sync):
            # write stream: identity rotations first (chunk-gated on the
            # input load), then the copy rotations as VectorE finishes them
            for r in ident:
                for c in range(C):
                    fsl = slice(c * FDC, (c + 1) * FDC)
                    sync.wait_ge(rd_sem, 16 * (c + 1))
                    sync.dma_start(
                        out.ap()[:, r][:, :, fsl], x_t[:, fsl]
                    ).then_inc(wr_sem, 16)
            for k, r in enumerate(copies):
                sync.wait_ge(cp_sem, k + 1)
                sync.dma_start(out.ap()[:, r], y_t[k % NB][:]).then_inc(
                    wr_sem, 16
                )
            sync.wait_ge(wr_sem, 16 * n_wr)

        def vector_body(vector):
            for k, r in enumerate(copies):
                if k >= NB:
                    # wait for the out-DMA that last read this y buffer
                    vector.wait_ge(wr_sem, 16 * (n_id * C + (k - NB) + 1))
                yt = y_t[k % NB]
                if k == 0:
                    # chunk-gated so copies start while the input streams in
                    for c in range(C):
                        vector.wait_ge(rd_sem, 16 * (c + 1))
                        _emit_rotation_copies(
                            vector, rot_plans[r], x_t, yt, cp_sem,
                            c * ILC, (c + 1) * ILC, last=(c == C - 1),
                        )
                else:
                    _emit_rotation_copies(
                        vector, rot_plans[r], x_t, yt, cp_sem, 0, IL, last=True
                    )

        if copies:
            block.vector(vector_body)

    return nc


def kernel(input, indices):
    from concourse.bass_utils import run_bass_kernel_spmd

    input = np.ascontiguousarray(np.asarray(input), dtype=np.float32)
    indices = np.asarray(indices)
    assert input.shape == (O, I, NORI, KH, KW), input.shape
    idx = indices.reshape(E, R).astype(np.int64) - 1
    inv = np.argsort(idx, axis=0, kind="stable")

    key = inv.tobytes()
    if key not in _cache:
        _cache[key] = _build(inv)
    nc = _cache[key]

    xs = input.reshape(O, I * E)
    in_maps = [
        {"input": np.ascontiguousarray(xs[c * O_SH : (c + 1) * O_SH]).reshape(P, FD)}
        for c in range(NCORES)
    ]
    res = run_bass_kernel_spmd(nc, in_maps, core_ids=list(range(NCORES)))
    parts = [res.results[c]["out"].reshape(O_SH, R, I, E) for c in range(NCORES)]
    full = np.concatenate(parts, axis=0)           # [O, R, I, E]
    return full.reshape(O * R, I * NORI, KH, KW)

